# revision 49
# baseline (speedup 1.0000x reference)
"""Trainium2 Bass kernel for causal multi-head attention.

Problem: B=2, S=2048, D=1024, H=16 heads (head_dim=64), fp32.
  y = softmax(causal(x@wq @ (x@wk)^T / sqrt(64))) @ (x@wv) @ wo + bo

Sharding (8 NeuronCores): 2 batches x 4 head-groups (4 heads each).
Each core computes, for its batch b and its 4 heads:
  - Q^T, K^T in [j, t] layout and V in [t, j] layout (j = 256 head cols)
  - scores^T[k, q] = K^T.T-free matmul, exp (scale=1/8, no max-sub --
    scores are ~N(0,1) so fp32 exp is safe), causal mask, then
    ctx^T[hd, q] with an appended ones-column giving softmax sums for free
  - per-q normalization via reciprocal + gpsimd partition-broadcast
  - partial y = ctx^T.T @ wo_slice  (row-shard of wo)
Host sums the 4 partials per batch and adds bo.

v2 scheduling notes (Tile's scheduler is dataflow + priority-heap, not
program order):
  - attention chain (scores/exp/mask/ctx/normalize) is wrapped in
    tc.high_priority so it always wins ready-ties on every engine;
    projections of later blocks and the out-projection act as PE
    fillers that keep the Tensor engine dense (max p-state).
  - PSUM tags are separated: "sc" (scores, 2 bufs x 2 banks),
    "cx" (ctx accumulators, 2 bufs x 1 bank), "pj" (projection +
    out-proj, 2 bufs x 1 bank) = 8 banks total, so fillers never
    contend with the attention chain for psum slots.
  - out-proj psum->sbuf copies run on gpsimd (Pool), keeping the
    Activation engine exp-only (~75us of exp is the pacing resource).
  - tri mask is materialized [P, 2, P] bf16 (no stride-0 broadcast) so
    the DVE multiply hits the fast 16-bit path.
  - xT is DMA'd in 4 column stripes so the first projection can start
    after ~1MB instead of ~4MB of input traffic.

Matmul operands are bf16 (host-cast); accumulation is fp32 in PSUM.
"""

import numpy as np

B, S, D, H = 2, 2048, 1024, 16
HD = 64          # head dim
NCORES = 8
HG = 4           # heads per core
JW = HG * HD     # 256: per-core head columns
P = 128
DC = D // P      # 8 contraction chunks for projections
TCB = S // 512   # 4: 512-token blocks
NT = S // P      # 16: 128-token chunks
CW = HD + 2      # 66: Vg per-head group width: [ones, V(64), pad]

_CACHE = {}


def _build_nc():
    import concourse.tile as tile
    from concourse import bacc, mybir

    f32 = mybir.dt.float32
    f32r = mybir.dt.bfloat16  # matmul operand dtype (fp32 accum in PSUM)
    EXP = mybir.ActivationFunctionType.Exp
    # priority tiers (bigger offset = earlier to the scheduler):
    ATT = 10_000_000        # attention chain (scores/exp/ctx/normalize)
    PRI_DRAIN = 20_000_000  # proj psum drain copies (keep fillers flowing)
    PRI_MASK = 30_000_000   # diag mask muls (gate ctx directly)

    nc = bacc.Bacc(None, target_bir_lowering=False)

    # All inputs are host-prearranged to partition-major layouts so every
    # DMA is 128 fat contiguous descriptors (tiny descriptors clog the
    # DMA queues for tens of us).
    # xT: [P, stripe, dc, 512], weights: [P, dc, JW], wo: [P, 2, D]
    xT_h = nc.dram_tensor("xT", [P, TCB, DC, 512], f32r, kind="ExternalInput")
    wq_h = nc.dram_tensor("wq", [P, DC, JW], f32r, kind="ExternalInput")
    wk_h = nc.dram_tensor("wk", [P, DC, JW], f32r, kind="ExternalInput")
    wv_h = nc.dram_tensor("wv", [P, DC, JW], f32r, kind="ExternalInput")
    wo_h = nc.dram_tensor("wo", [P, 2, D], f32r, kind="ExternalInput")
    tri_h = nc.dram_tensor("tri", [P, 2 * P], f32r, kind="ExternalInput")
    y_h = nc.dram_tensor("y", [S, D], f32, kind="ExternalOutput")

    with tile.TileContext(nc) as tc:
        with (
            tc.tile_pool(name="const", bufs=1) as cp,
            tc.tile_pool(name="work", bufs=2) as wp,
            tc.tile_pool(name="psum", bufs=2, space="PSUM") as pp,
        ):
            # ---- resident SBUF tensors ----
            xT_sb = cp.tile([P, TCB, DC, 512], f32r, name="xT_sb")   # 32KB/part
            wq_sb = cp.tile([P, DC, JW], f32r, name="wq_sb")         # 4KB
            wk_sb = cp.tile([P, DC, JW], f32r, name="wk_sb")
            wv_sb = cp.tile([P, DC, JW], f32r, name="wv_sb")
            wo_sb = cp.tile([P, 2, D], f32r, name="wo_sb")           # 4KB
            tri_sb = cp.tile([P, 2, P], f32r, name="tri_sb")
            QT = cp.tile([P, 2, S], f32r, name="QT")                 # 8KB
            KT = cp.tile([P, 2, S], f32r, name="KT")
            Vg = cp.tile([P, NT, HG * CW], f32r, name="Vg")
            ctxn = cp.tile([P, 2, S], f32r, name="ctxn")             # 8KB

            # ---- input DMAs, ordered so the j4=0 pipeline can start
            #      as early as possible; all fat contiguous descriptors ----
            # wk/stripe0 split into dc-halves so the first K-proj psum
            # chain can start after ~0.75MB instead of 1.5MB of input
            nc.sync.dma_start(out=wk_sb[:, 0:4, :], in_=wk_h[:, 0:4, :])
            nc.sync.dma_start(out=xT_sb[:, 0, 0:4, :], in_=xT_h[:, 0, 0:4, :])
            nc.sync.dma_start(out=wk_sb[:, 4:8, :], in_=wk_h[:, 4:8, :])
            nc.sync.dma_start(out=xT_sb[:, 0, 4:8, :], in_=xT_h[:, 0, 4:8, :])
            nc.sync.dma_start(out=wq_sb, in_=wq_h[:])
            nc.sync.dma_start(
                out=tri_sb, in_=tri_h[:].rearrange("p (j q) -> p j q", j=2)
            )
            nc.sync.dma_start(out=wv_sb, in_=wv_h[:])
            for st in range(1, TCB):
                nc.sync.dma_start(out=xT_sb[:, st], in_=xT_h[:, st])
            nc.sync.dma_start(out=wo_sb, in_=wo_h[:])
            # ones columns of Vg (col HD of each per-head 66-group) via
            # a single strided gpsimd memset instead of a scatter-DMA
            nc.gpsimd.memset(
                Vg[:, :, :].rearrange("p t (h c) -> p t h c", c=CW)[:, :, :, HD],
                1.0,
            )

            # ---- PE warm-up: zero matmuls at natural (earliest) priority
            #      run during the input-DMA wait so the PE reaches max
            #      p-state before the first projection (p-state ramp costs
            #      ~2x-4x for ~3us after any idle). Sized to finish right
            #      as wk/xT land; the psum slot is released immediately.
            warm_sb = cp.tile([P, 512], f32r, name="warm_sb")
            nc.vector.memset(warm_sb, 0.0)
            pw = pp.tile([P, 512], f32, tag="sc", bufs=2, name="pw")
            NWARM = 20
            for wi in range(NWARM):
                nc.tensor.matmul(
                    pw,
                    lhsT=warm_sb[:, 0:P],
                    rhs=warm_sb,
                    start=(wi == 0),
                    stop=(wi == NWARM - 1),
                )

            def out_proj(j4, hot=False):
                # ---- output projection for q-block j4 (filler priority).
                #      Issued one iteration late so the shared "pj" psum
                #      ring never makes a later block's projection wait on
                #      this block's out-proj drain. `hot` = tail mode: top
                #      priority + per-half DMAs to minimize drain latency.
                from contextlib import nullcontext
                prio = (lambda: tc.high_priority(offset=PRI_MASK)) if hot \
                    else nullcontext
                for tb4 in range(4):
                    tbo = 4 * j4 + tb4
                    ysb = wp.tile([P, D], f32, tag="y", bufs=2, name="ysb")
                    for nn2 in range(2):
                        py = pp.tile([P, 512], f32, tag="pj", bufs=2,
                                     name="py")
                        with prio():
                            for jc in range(2):
                                nc.tensor.matmul(
                                    py,
                                    lhsT=ctxn[:, jc, tbo * P:(tbo + 1) * P],
                                    rhs=wo_sb[:, jc,
                                              nn2 * 512:(nn2 + 1) * 512],
                                    start=(jc == 0),
                                    stop=(jc == 1),
                                )
                            # split psum evacuation across ACT and DVE: ACT
                            # is idle exactly when out-proj runs
                            if nn2 == 0:
                                nc.scalar.copy(
                                    out=ysb[:, nn2 * 512:(nn2 + 1) * 512],
                                    in_=py,
                                )
                            else:
                                nc.vector.tensor_copy(
                                    out=ysb[:, nn2 * 512:(nn2 + 1) * 512],
                                    in_=py,
                                )
                        if hot:
                            nc.sync.dma_start(
                                out=y_h[tbo * P:(tbo + 1) * P,
                                        nn2 * 512:(nn2 + 1) * 512],
                                in_=ysb[:, nn2 * 512:(nn2 + 1) * 512],
                            )
                    if not hot:
                        nc.sync.dma_start(
                            out=y_h[tbo * P:(tbo + 1) * P, :], in_=ysb
                        )

            def qk_proj(tb):
                # ---- Q/K projection for token stripe tb (filler
                #      priority: hides under earlier blocks' attention) ----
                for w_sb, dst in ((wk_sb, KT), (wq_sb, QT)):
                    for jc in range(2):
                        pq = pp.tile([P, 512], f32, tag="pj", bufs=2, name="pq")
                        for dc in range(DC):
                            nc.tensor.matmul(
                                pq,
                                lhsT=w_sb[:, dc, jc * P:(jc + 1) * P],
                                rhs=xT_sb[:, tb, dc, :],
                                start=(dc == 0),
                                stop=(dc == DC - 1),
                            )
                        # drain copy above the attention chain: the proj
                        # filler stream must never stall on a busy DVE, or
                        # the PE runs dry and drops out of max p-state
                        with tc.high_priority(offset=PRI_DRAIN):
                            nc.vector.tensor_copy(
                                out=dst[:, jc, tb * 512:(tb + 1) * 512],
                                in_=pq,
                            )

            qk_proj(0)
            for j4 in range(TCB):
                for tv in range(4 * j4, 4 * j4 + 4):
                    pv = pp.tile([P, JW], f32, tag="pj", bufs=2, name="pv")
                    for dc in range(DC):
                        nc.tensor.matmul(
                            pv,
                            lhsT=xT_sb[:, tv // 4, dc,
                                       (tv % 4) * P:(tv % 4 + 1) * P],
                            rhs=wv_sb[:, dc, :],
                            start=(dc == 0),
                            stop=(dc == DC - 1),
                        )
                    with tc.high_priority(offset=PRI_DRAIN):
                        nc.vector.tensor_copy(
                            out=Vg[:, tv, :].rearrange(
                                "p (h c) -> p h c", c=CW
                            )[:, :, 0:HD],
                            in_=pv.rearrange("p (h c) -> p h c", c=HD),
                        )

                # Q/K projection of the NEXT stripe, hoisted so its psum
                # drain (QT/KT copies) completes during attention(j4) and
                # the j4->j4+1 transition never waits on it.
                if j4 < TCB - 1:
                    qk_proj(j4 + 1)

                # ---- attention for q-block j4 (high priority chain) ----
                with tc.high_priority(offset=ATT):
                    for pr in range(2):        # pair index = chunk index (h//2)
                        qs = slice(j4 * 512, (j4 + 1) * 512)
                        nchunks = 4 * j4 + 4
                        pctx = []
                        for hh in range(2):
                            pc = pp.tile([HD + 1, 512], f32, tag="cx", bufs=2,
                                         name=f"pc{hh}")
                            pctx.append(pc)
                        for c in range(nchunks):
                            # columns [0, o) are fully masked for this k-chunk:
                            # skip them in scores, exp and ctx entirely.
                            o = P * (c - 4 * j4) if c >= 4 * j4 else 0
                            ps2 = pp.tile([P, 2, 512], f32, tag="sc", bufs=2,
                                          name="ps2")
                            for hh in range(2):
                                bp = HD * hh   # partition base for this head
                                nc.tensor.matmul(
                                    ps2[:, hh, o:512],
                                    lhsT=KT[bp:bp + HD, pr, c * P:(c + 1) * P],
                                    rhs=QT[bp:bp + HD, pr,
                                           j4 * 512 + o:(j4 + 1) * 512],
                                    start=True,
                                    stop=True,
                                )
                            et = wp.tile([P, 2, 512], f32r, tag="exp", bufs=10,
                                         name="et")
                            nc.scalar.activation(
                                out=et[:, :, o:512], in_=ps2[:, :, o:512],
                                func=EXP, scale=0.125,
                            )
                            if c >= 4 * j4:
                                # per-head 2D contiguous muls (DVE fast
                                # mode); highest priority: directly gates
                                # the ctx matmul on the PE
                                with tc.high_priority(offset=PRI_MASK):
                                    for hh in range(2):
                                        nc.vector.tensor_mul(
                                            out=et[:, hh, o:o + P],
                                            in0=et[:, hh, o:o + P],
                                            in1=tri_sb[:, hh, :],
                                        )
                            for hh in range(2):
                                h = 2 * pr + hh
                                nc.tensor.matmul(
                                    pctx[hh][:, o:512],
                                    lhsT=Vg[:, c, h * CW:h * CW + HD + 1],
                                    rhs=et[:, hh, o:512],
                                    start=(c == 0),
                                    stop=(c == nchunks - 1),
                                )
                        # normalize: ctx^T[hd, q] * (1/sum[q]); sums ride at
                        # psum partition HD=64 (ones column is last in Vg).
                        # Evacuate the psum bank IMMEDIATELY (two copies at
                        # top priority) so ctx of the next pr-group can
                        # start accumulating; the reciprocal/broadcast/mul
                        # tail then runs off the critical path from SBUF.
                        # HW quirk (verified on-device): custom-DVE ops and
                        # partition_broadcast misread sources at partition
                        # base 64 -- hence the separate base-0 sums hop.
                        # op-major issue order so the two hh chains overlap
                        # across DVE (recip/mul) and Pool (broadcast)
                        ctxus, rts, rbcs = [], [], []
                        for hh in range(2):
                            pc = pctx[hh]
                            ctxu = wp.tile([HD, 512], f32r, tag="ctxu",
                                           bufs=4, name="ctxu")
                            sums = wp.tile([1, 512], f32, tag="rt", bufs=4,
                                           name="sums")
                            with tc.high_priority(offset=PRI_MASK):
                                # ctxu on ACT (idle at pr boundaries),
                                # sums on DVE: bank freed in ~0.7us
                                nc.scalar.copy(out=ctxu, in_=pc[0:HD, :])
                                nc.vector.tensor_copy(out=sums,
                                                      in_=pc[HD:HD + 1, :])
                            rt = wp.tile([1, 512], f32, tag="rt", bufs=4,
                                         name="rt")
                            nc.vector.reciprocal_approx_fast(out=rt, in_=sums)
                            ctxus.append(ctxu)
                            rts.append(rt)
                        for hh in range(2):
                            rbc = wp.tile([HD, 512], f32, tag="rbc", bufs=4,
                                          name="rbc")
                            nc.gpsimd.partition_broadcast(
                                rbc[:, :], rts[hh][0:1, :], channels=HD
                            )
                            rbcs.append(rbc)
                        for hh in range(2):
                            nc.vector.tensor_mul(
                                out=ctxn[HD * hh:HD * (hh + 1), pr, qs],
                                in0=ctxus[hh],
                                in1=rbcs[hh][:, :],
                            )

                # out-proj of the previous q-block: created after this
                # block's attention so the "pj" ring order per cycle is
                # [pv(j4), pq(j4+1), py(j4-1)] -- projections never wait
                # on an out-proj drain.
                if j4 >= 1:
                    out_proj(j4 - 1)
            out_proj(TCB - 1, hot=True)

    nc.compile()
    return nc


def get_nc(debug=False):
    key = ("nc", debug)
    if key not in _CACHE:
        _CACHE[key] = _build_nc()
    return _CACHE[key]


def make_in_maps(x, wq, wk, wv, wo):
    import ml_dtypes
    bf16 = ml_dtypes.bfloat16
    x = np.asarray(x, dtype=np.float32)
    wq = np.asarray(wq, dtype=np.float32).astype(bf16)
    wk = np.asarray(wk, dtype=np.float32).astype(bf16)
    wv = np.asarray(wv, dtype=np.float32).astype(bf16)
    wo = np.asarray(wo, dtype=np.float32).astype(bf16)
    # tri[k, q] = 1 if k <= q else 0   (causal keep-mask for diagonal blocks)
    # materialized twice along the free dim (one copy per head of the pair)
    tri1 = np.tril(np.ones((P, P), dtype=np.float32)).T
    tri = np.concatenate([tri1, tri1], axis=1).astype(bf16)

    def pmajor_w(w):
        # [D, JW] -> [P, DC, JW]: row d = dc*P + p
        return np.ascontiguousarray(
            np.asarray(w).reshape(DC, P, JW).transpose(1, 0, 2)
        )

    # x[b]: [S, D] -> xT [D, S] -> [P, stripe, dc, 512]
    xTs = []
    for b in range(B):
        xT = x[b].T.astype(bf16)                       # [D, S]
        xT = xT.reshape(DC, P, TCB, 512)               # [dc, p, st, t]
        xTs.append(np.ascontiguousarray(xT.transpose(1, 2, 0, 3)))
    in_maps = []
    for core in range(NCORES):
        b, g = core // HG, core % HG
        jsl = slice(g * JW, (g + 1) * JW)
        in_maps.append({
            "xT": xTs[b],
            "wq": pmajor_w(wq[:, jsl]),
            "wk": pmajor_w(wk[:, jsl]),
            "wv": pmajor_w(wv[:, jsl]),
            # wo [JW, D] -> [P, 2, D]: row j = ch*P + p
            "wo": np.ascontiguousarray(
                np.asarray(wo[jsl, :]).reshape(2, P, D).transpose(1, 0, 2)
            ),
            "tri": tri,
        })
    return in_maps


def combine_outputs(results, bo):
    bo = np.asarray(bo, dtype=np.float32)
    y = np.zeros((B, S, D), dtype=np.float32)
    for core in range(NCORES):
        y[core // HG] += results[core]["y"]
    y += bo[None, None, :]
    return y


def kernel(x, wq, wk, wv, wo, bo):
    from concourse.bass_utils import run_bass_kernel_spmd

    nc = get_nc()
    in_maps = make_in_maps(x, wq, wk, wv, wo)
    res = run_bass_kernel_spmd(nc, in_maps, core_ids=list(range(NCORES)))
    return combine_outputs(res.results, bo)


# revision 51
# speedup vs baseline: 1.1769x; 1.1769x over previous
"""Trainium2 Bass kernel for causal multi-head attention.

Problem: B=2, S=2048, D=1024, H=16 heads (head_dim=64), fp32.
  y = softmax(causal(x@wq @ (x@wk)^T / sqrt(64))) @ (x@wv) @ wo + bo

Sharding (8 NeuronCores): 2 batches x 4 head-groups (4 heads each).
Each core computes, for its batch b and its 4 heads:
  - Q^T, K^T in [j, t] layout and V in [t, j] layout (j = 256 head cols)
  - scores^T[k, q] = K^T.T-free matmul, exp (scale=1/8, no max-sub --
    scores are ~N(0,1) so fp32 exp is safe), causal mask, then
    ctx^T[hd, q] with an appended ones-column giving softmax sums for free
  - per-q normalization via reciprocal + gpsimd partition-broadcast
  - partial y = ctx^T.T @ wo_slice  (row-shard of wo)
Host sums the 4 partials per batch and adds bo.

v2 scheduling notes (Tile's scheduler is dataflow + priority-heap, not
program order):
  - attention chain (scores/exp/mask/ctx/normalize) is wrapped in
    tc.high_priority so it always wins ready-ties on every engine;
    projections of later blocks and the out-projection act as PE
    fillers that keep the Tensor engine dense (max p-state).
  - PSUM tags are separated: "sc" (scores, 2 bufs x 2 banks),
    "cx" (ctx accumulators, 2 bufs x 1 bank), "pj" (projection +
    out-proj, 2 bufs x 1 bank) = 8 banks total, so fillers never
    contend with the attention chain for psum slots.
  - out-proj psum->sbuf copies run on gpsimd (Pool), keeping the
    Activation engine exp-only (~75us of exp is the pacing resource).
  - tri mask is materialized [P, 2, P] bf16 (no stride-0 broadcast) so
    the DVE multiply hits the fast 16-bit path.
  - xT is DMA'd in 4 column stripes so the first projection can start
    after ~1MB instead of ~4MB of input traffic.

Matmul operands are bf16 (host-cast); accumulation is fp32 in PSUM.
"""

import numpy as np

B, S, D, H = 2, 2048, 1024, 16
HD = 64          # head dim
NCORES = 8
HG = 4           # heads per core
JW = HG * HD     # 256: per-core head columns
P = 128
DC = D // P      # 8 contraction chunks for projections
TCB = S // 512   # 4: 512-token blocks
NT = S // P      # 16: 128-token chunks
CW = HD + 2      # 66: Vg per-head group width: [ones, V(64), pad]

_CACHE = {}


def _build_nc():
    import concourse.tile as tile
    from concourse import bacc, mybir

    f32 = mybir.dt.float32
    f32r = mybir.dt.bfloat16  # matmul operand dtype (fp32 accum in PSUM)
    EXP = mybir.ActivationFunctionType.Exp
    # priority tiers (bigger offset = earlier to the scheduler):
    ATT = 10_000_000        # attention chain (scores/exp/ctx/normalize)
    PRI_DRAIN = 20_000_000  # proj psum drain copies (keep fillers flowing)
    PRI_MASK = 30_000_000   # diag mask muls (gate ctx directly)

    nc = bacc.Bacc(None, target_bir_lowering=False)

    # All inputs are host-prearranged to partition-major layouts so every
    # DMA is 128 fat contiguous descriptors (tiny descriptors clog the
    # DMA queues for tens of us).
    # xT: [P, stripe, dc, 512], weights: [P, dc, JW], wo: [P, 2, D]
    xT_h = nc.dram_tensor("xT", [P, TCB, DC, 512], f32r, kind="ExternalInput")
    wq_h = nc.dram_tensor("wq", [P, DC, JW], f32r, kind="ExternalInput")
    wk_h = nc.dram_tensor("wk", [P, DC, JW], f32r, kind="ExternalInput")
    wv_h = nc.dram_tensor("wv", [P, DC, JW], f32r, kind="ExternalInput")
    wo_h = nc.dram_tensor("wo", [P, 2, D], f32r, kind="ExternalInput")
    tri_h = nc.dram_tensor("tri", [P, 2 * P], f32r, kind="ExternalInput")
    y_h = nc.dram_tensor("y", [S, D], f32, kind="ExternalOutput")

    with tile.TileContext(nc) as tc:
        with (
            tc.tile_pool(name="const", bufs=1) as cp,
            tc.tile_pool(name="work", bufs=2) as wp,
            tc.tile_pool(name="psum", bufs=2, space="PSUM") as pp,
        ):
            # ---- resident SBUF tensors ----
            xT_sb = cp.tile([P, TCB, DC, 512], f32r, name="xT_sb")   # 32KB/part
            wq_sb = cp.tile([P, DC, JW], f32r, name="wq_sb")         # 4KB
            wk_sb = cp.tile([P, DC, JW], f32r, name="wk_sb")
            wv_sb = cp.tile([P, DC, JW], f32r, name="wv_sb")
            wo_sb = cp.tile([P, 2, D], f32r, name="wo_sb")           # 4KB
            tri_sb = cp.tile([P, 2, P], f32r, name="tri_sb")
            QT = cp.tile([P, 2, S], f32r, name="QT")                 # 8KB
            KT = cp.tile([P, 2, S], f32r, name="KT")
            Vg = cp.tile([P, NT, HG * CW], f32r, name="Vg")
            ctxn = cp.tile([P, 2, S], f32r, name="ctxn")             # 8KB

            # ---- input DMAs, ordered so the j4=0 pipeline can start
            #      as early as possible; all fat contiguous descriptors ----
            # wk/stripe0 split into dc-halves: the K-proj psum chain
            # consumes dc in order, so it can start after ~0.75MB of
            # input instead of 1.5MB (pipelines DMA with the matmuls)
            nc.sync.dma_start(out=wk_sb[:, 0:4, :], in_=wk_h[:, 0:4, :])
            nc.sync.dma_start(out=xT_sb[:, 0, 0:4, :], in_=xT_h[:, 0, 0:4, :])
            nc.sync.dma_start(out=wk_sb[:, 4:8, :], in_=wk_h[:, 4:8, :])
            nc.sync.dma_start(out=xT_sb[:, 0, 4:8, :], in_=xT_h[:, 0, 4:8, :])
            nc.sync.dma_start(out=wq_sb, in_=wq_h[:])
            nc.sync.dma_start(
                out=tri_sb, in_=tri_h[:].rearrange("p (j q) -> p j q", j=2)
            )
            nc.sync.dma_start(out=wv_sb, in_=wv_h[:])
            for st in range(1, TCB):
                nc.sync.dma_start(out=xT_sb[:, st], in_=xT_h[:, st])
            nc.sync.dma_start(out=wo_sb, in_=wo_h[:])
            # ones columns of Vg (col HD of each per-head 66-group) via
            # a single strided gpsimd memset instead of a scatter-DMA
            nc.gpsimd.memset(
                Vg[:, :, :].rearrange("p t (h c) -> p t h c", c=CW)[:, :, :, HD],
                1.0,
            )

            # ---- PE warm-up: zero matmuls at natural (earliest) priority
            #      run during the input-DMA wait so the PE reaches max
            #      p-state before the first projection (p-state ramp costs
            #      ~2x-4x for ~3us after any idle). Sized to finish right
            #      as wk/xT land; the psum slot is released immediately.
            warm_sb = cp.tile([P, 512], f32r, name="warm_sb")
            nc.vector.memset(warm_sb, 0.0)
            pw = pp.tile([P, 512], f32, tag="sc", bufs=2, name="pw")
            NWARM = 20
            for wi in range(NWARM):
                nc.tensor.matmul(
                    pw,
                    lhsT=warm_sb[:, 0:P],
                    rhs=warm_sb,
                    start=(wi == 0),
                    stop=(wi == NWARM - 1),
                )

            def out_proj(j4, hot=False):
                # ---- output projection for q-block j4 (filler priority).
                #      Issued one iteration late so the shared "pj" psum
                #      ring never makes a later block's projection wait on
                #      this block's out-proj drain. `hot` = tail mode: top
                #      priority + per-half DMAs to minimize drain latency.
                from contextlib import nullcontext
                prio = (lambda: tc.high_priority(offset=PRI_MASK)) if hot \
                    else nullcontext
                for tb4 in range(4):
                    tbo = 4 * j4 + tb4
                    ysb = wp.tile([P, D], f32, tag="y", bufs=2, name="ysb")
                    for nn2 in range(2):
                        py = pp.tile([P, 512], f32, tag="pj", bufs=2,
                                     name="py")
                        with prio():
                            for jc in range(2):
                                nc.tensor.matmul(
                                    py,
                                    lhsT=ctxn[:, jc, tbo * P:(tbo + 1) * P],
                                    rhs=wo_sb[:, jc,
                                              nn2 * 512:(nn2 + 1) * 512],
                                    start=(jc == 0),
                                    stop=(jc == 1),
                                )
                            # split psum evacuation across ACT and DVE: ACT
                            # is idle exactly when out-proj runs
                            if nn2 == 0:
                                nc.scalar.copy(
                                    out=ysb[:, nn2 * 512:(nn2 + 1) * 512],
                                    in_=py,
                                )
                            else:
                                nc.vector.tensor_copy(
                                    out=ysb[:, nn2 * 512:(nn2 + 1) * 512],
                                    in_=py,
                                )
                        if hot:
                            nc.sync.dma_start(
                                out=y_h[tbo * P:(tbo + 1) * P,
                                        nn2 * 512:(nn2 + 1) * 512],
                                in_=ysb[:, nn2 * 512:(nn2 + 1) * 512],
                            )
                    if not hot:
                        nc.sync.dma_start(
                            out=y_h[tbo * P:(tbo + 1) * P, :], in_=ysb
                        )

            for j4 in range(TCB):
                tb = j4
                # ---- Q/K projection for this token stripe (filler
                #      priority: hides under earlier blocks' attention) ----
                for w_sb, dst in ((wk_sb, KT), (wq_sb, QT)):
                    for jc in range(2):
                        pq = pp.tile([P, 512], f32, tag="pj", bufs=2, name="pq")
                        for dc in range(DC):
                            nc.tensor.matmul(
                                pq,
                                lhsT=w_sb[:, dc, jc * P:(jc + 1) * P],
                                rhs=xT_sb[:, tb, dc, :],
                                start=(dc == 0),
                                stop=(dc == DC - 1),
                            )
                        # drain copy above the attention chain: the proj
                        # filler stream must never stall on a busy DVE, or
                        # the PE runs dry and drops out of max p-state
                        with tc.high_priority(offset=PRI_DRAIN):
                            nc.vector.tensor_copy(
                                out=dst[:, jc, tb * 512:(tb + 1) * 512],
                                in_=pq,
                            )
                # out-proj of block j4-2, created HERE so the "pj" psum
                # ring order is [pq(j4), py(j4-2), pv(j4)]: nothing the
                # attention chain needs ever waits on a py drain.
                if j4 >= 2:
                    out_proj(j4 - 2)
                for tv in range(4 * j4, 4 * j4 + 4):
                    pv = pp.tile([P, JW], f32, tag="pj", bufs=2, name="pv")
                    for dc in range(DC):
                        nc.tensor.matmul(
                            pv,
                            lhsT=xT_sb[:, tv // 4, dc,
                                       (tv % 4) * P:(tv % 4 + 1) * P],
                            rhs=wv_sb[:, dc, :],
                            start=(dc == 0),
                            stop=(dc == DC - 1),
                        )
                    with tc.high_priority(offset=PRI_DRAIN):
                        nc.vector.tensor_copy(
                            out=Vg[:, tv, :].rearrange(
                                "p (h c) -> p h c", c=CW
                            )[:, :, 0:HD],
                            in_=pv.rearrange("p (h c) -> p h c", c=HD),
                        )

                # ---- attention for q-block j4 (high priority chain) ----
                with tc.high_priority(offset=ATT):
                    for pr in range(2):        # pair index = chunk index (h//2)
                        qs = slice(j4 * 512, (j4 + 1) * 512)
                        nchunks = 4 * j4 + 4
                        pctx = []
                        for hh in range(2):
                            pc = pp.tile([HD + 1, 512], f32, tag="cx", bufs=2,
                                         name=f"pc{hh}")
                            pctx.append(pc)
                        for c in range(nchunks):
                            # columns [0, o) are fully masked for this k-chunk:
                            # skip them in scores, exp and ctx entirely.
                            o = P * (c - 4 * j4) if c >= 4 * j4 else 0
                            ps2 = pp.tile([P, 2, 512], f32, tag="sc", bufs=2,
                                          name="ps2")
                            for hh in range(2):
                                bp = HD * hh   # partition base for this head
                                nc.tensor.matmul(
                                    ps2[:, hh, o:512],
                                    lhsT=KT[bp:bp + HD, pr, c * P:(c + 1) * P],
                                    rhs=QT[bp:bp + HD, pr,
                                           j4 * 512 + o:(j4 + 1) * 512],
                                    start=True,
                                    stop=True,
                                )
                            et = wp.tile([P, 2, 512], f32r, tag="exp", bufs=10,
                                         name="et")
                            nc.scalar.activation(
                                out=et[:, :, o:512], in_=ps2[:, :, o:512],
                                func=EXP, scale=0.125,
                            )
                            if c >= 4 * j4:
                                # per-head 2D contiguous muls (DVE fast
                                # mode); highest priority: directly gates
                                # the ctx matmul on the PE
                                with tc.high_priority(offset=PRI_MASK):
                                    for hh in range(2):
                                        nc.vector.tensor_mul(
                                            out=et[:, hh, o:o + P],
                                            in0=et[:, hh, o:o + P],
                                            in1=tri_sb[:, hh, :],
                                        )
                            for hh in range(2):
                                h = 2 * pr + hh
                                nc.tensor.matmul(
                                    pctx[hh][:, o:512],
                                    lhsT=Vg[:, c, h * CW:h * CW + HD + 1],
                                    rhs=et[:, hh, o:512],
                                    start=(c == 0),
                                    stop=(c == nchunks - 1),
                                )
                        # normalize: ctx^T[hd, q] * (1/sum[q]); sums ride at
                        # psum partition HD=64 (ones column is last in Vg).
                        # Evacuate the psum bank IMMEDIATELY (two copies at
                        # top priority) so ctx of the next pr-group can
                        # start accumulating; the reciprocal/broadcast/mul
                        # tail then runs off the critical path from SBUF.
                        # HW quirk (verified on-device): custom-DVE ops and
                        # partition_broadcast misread sources at partition
                        # base 64 -- hence the separate base-0 sums hop.
                        # op-major issue order so the two hh chains overlap
                        # across DVE (recip/mul) and Pool (broadcast)
                        ctxus, rts, rbcs = [], [], []
                        for hh in range(2):
                            pc = pctx[hh]
                            ctxu = wp.tile([HD, 512], f32r, tag="ctxu",
                                           bufs=4, name="ctxu")
                            sums = wp.tile([1, 512], f32, tag="rt", bufs=4,
                                           name="sums")
                            with tc.high_priority(offset=PRI_MASK):
                                # ctxu on ACT (idle at pr boundaries),
                                # sums on DVE: bank freed in ~0.7us
                                nc.scalar.copy(out=ctxu, in_=pc[0:HD, :])
                                nc.vector.tensor_copy(out=sums,
                                                      in_=pc[HD:HD + 1, :])
                            rt = wp.tile([1, 512], f32, tag="rt", bufs=4,
                                         name="rt")
                            nc.vector.reciprocal_approx_fast(out=rt, in_=sums)
                            ctxus.append(ctxu)
                            rts.append(rt)
                        for hh in range(2):
                            rbc = wp.tile([HD, 512], f32, tag="rbc", bufs=4,
                                          name="rbc")
                            nc.gpsimd.partition_broadcast(
                                rbc[:, :], rts[hh][0:1, :], channels=HD
                            )
                            rbcs.append(rbc)
                        for hh in range(2):
                            nc.vector.tensor_mul(
                                out=ctxn[HD * hh:HD * (hh + 1), pr, qs],
                                in0=ctxus[hh],
                                in1=rbcs[hh][:, :],
                            )

            out_proj(TCB - 2)
            out_proj(TCB - 1, hot=True)

    nc.compile()
    return nc


def get_nc(debug=False):
    key = ("nc", debug)
    if key not in _CACHE:
        _CACHE[key] = _build_nc()
    return _CACHE[key]


def make_in_maps(x, wq, wk, wv, wo):
    import ml_dtypes
    bf16 = ml_dtypes.bfloat16
    x = np.asarray(x, dtype=np.float32)
    wq = np.asarray(wq, dtype=np.float32).astype(bf16)
    wk = np.asarray(wk, dtype=np.float32).astype(bf16)
    wv = np.asarray(wv, dtype=np.float32).astype(bf16)
    wo = np.asarray(wo, dtype=np.float32).astype(bf16)
    # tri[k, q] = 1 if k <= q else 0   (causal keep-mask for diagonal blocks)
    # materialized twice along the free dim (one copy per head of the pair)
    tri1 = np.tril(np.ones((P, P), dtype=np.float32)).T
    tri = np.concatenate([tri1, tri1], axis=1).astype(bf16)

    def pmajor_w(w):
        # [D, JW] -> [P, DC, JW]: row d = dc*P + p
        return np.ascontiguousarray(
            np.asarray(w).reshape(DC, P, JW).transpose(1, 0, 2)
        )

    # x[b]: [S, D] -> xT [D, S] -> [P, stripe, dc, 512]
    xTs = []
    for b in range(B):
        xT = x[b].T.astype(bf16)                       # [D, S]
        xT = xT.reshape(DC, P, TCB, 512)               # [dc, p, st, t]
        xTs.append(np.ascontiguousarray(xT.transpose(1, 2, 0, 3)))
    in_maps = []
    for core in range(NCORES):
        b, g = core // HG, core % HG
        jsl = slice(g * JW, (g + 1) * JW)
        in_maps.append({
            "xT": xTs[b],
            "wq": pmajor_w(wq[:, jsl]),
            "wk": pmajor_w(wk[:, jsl]),
            "wv": pmajor_w(wv[:, jsl]),
            # wo [JW, D] -> [P, 2, D]: row j = ch*P + p
            "wo": np.ascontiguousarray(
                np.asarray(wo[jsl, :]).reshape(2, P, D).transpose(1, 0, 2)
            ),
            "tri": tri,
        })
    return in_maps


def combine_outputs(results, bo):
    bo = np.asarray(bo, dtype=np.float32)
    y = np.zeros((B, S, D), dtype=np.float32)
    for core in range(NCORES):
        y[core // HG] += results[core]["y"]
    y += bo[None, None, :]
    return y


def kernel(x, wq, wk, wv, wo, bo):
    from concourse.bass_utils import run_bass_kernel_spmd

    nc = get_nc()
    in_maps = make_in_maps(x, wq, wk, wv, wo)
    res = run_bass_kernel_spmd(nc, in_maps, core_ids=list(range(NCORES)))
    return combine_outputs(res.results, bo)
